# revision 61
# baseline (speedup 1.0000x reference)
"""Longformer block on 8 TRN2 NeuronCores (Bass/Tile, SPMD).

Sharding: data-parallel over (batch, sequence): core c -> batch c//4, token
chunk (c%4)*512..+512. Weights replicated. Everything on-chip stays in
transposed [D, token] layout so LN/residual/matmuls need no device transposes
(host pre-transposes x; LN stats via ones-vector matmuls on PE).

QKV, out-proj and the PV attention matmuls run as fp8e4 DoubleRow matmuls
(weights host-quantized x2048, activations device-quantized x16); FFN stays
bf16 (fp8 there costs ~1.2e-2 rel err per site). FFN hidden stays in SBUF
(no DRAM spill); W2 is fully resident so FFN2 runs m-outer with per-m
evacuation (no drain tail).

Attention: banded causal window (halo of 128 tokens recomputed locally) + the
token-0 global column as a 257th score column. The one global *row* (token
T-1 attends everything) is computed via per-core exp-sum partials over each
core's own K/V slice, combined with a tiny in-kernel AllReduce (each core
deposits its partial into its batch's block, scaled by 0/1 flag inputs), and
patched into the owning core's output column with copy_predicated.
"""

import numpy as np
import ml_dtypes

import concourse.bass as bass
import concourse.mybir as mybir
import concourse.tile as tile
from concourse.masks import make_identity
from concourse.bass_utils import run_bass_kernel_spmd

F32 = mybir.dt.float32
BF16 = mybir.dt.bfloat16
FP8 = mybir.dt.float8e4
AF = mybir.ActivationFunctionType
ALU = mybir.AluOpType
AX = mybir.AxisListType
DR = mybir.MatmulPerfMode.DoubleRow

D = 1024
H = 16
HD = 64
T = 2048
B = 2
CHUNK = 512
HALO = 128
NSLOT = 768          # [halo 128 | own 512 | t0 | t2047 | pad]
NKV = 641            # slots 0..640 hold K/V (640 = token0); 641 = q2047 src
NQB = 4
WIN = 256
NEG = -1e30
EPS = 1e-5
N_CORES = 8
SW = 2048.0          # host weight scale for fp8 (Wq/Wk/Wv/Wo)
SH = 16.0            # activation scale for fp8 (hT, OT, V8, P)
SKIP_CC = [False]   # set kernel.SKIP_CC[0]=True to build without the
                    # collective (TimelineSim is single-core only)
PHASE_MARKS = []    # (phase_name, first_inst_id) filled during _emit


def _mark(nc, name):
    PHASE_MARKS.append((name, set(nc.inst_map.keys())))

# ---------------------------------------------------------------- bir fix ---

_waitfix_ctr = [0]


def _split_multiwaits(nc):
    """This container's walrus accepts ONE sync-wait per instruction; Tile
    attaches several. Hoist extras onto NoOps just before each instruction
    (Tile sems are monotonic within a context, so sequential waits are
    equivalent)."""
    n = 0
    for func in nc.m.functions:
        for bb in func.blocks:
            out = []
            changed = False
            for inst in bb.instructions:
                si = inst.sync_info
                if si is not None and len(si.on_wait) > 1:
                    waits = list(si.on_wait)
                    keep = [w for w in waits
                            if getattr(w, "wait_mode", "") not in
                            ("sem-ge-imm", "sem-ge-reg")]
                    if keep:
                        hoist = [w for w in waits if w not in keep]
                        last = keep
                    else:
                        hoist, last = waits[:-1], [waits[-1]]
                    for w in hoist:
                        _waitfix_ctr[0] += 1
                        nop = mybir.InstNoOp(name=f"I-waitfix-{_waitfix_ctr[0]}")
                        nop.engine = inst.engine
                        nop.sync_info = mybir.SyncInfo(on_wait=[w], on_update=[])
                        out.append(nop)
                        n += 1
                    si.on_wait = last
                    changed = True
                out.append(inst)
            if changed:
                bb.instructions[:] = out
    return n

# ------------------------------------------------------------ host helpers --


def _make_x_ext(x, c):
    b, j = divmod(c, 4)
    start = j * CHUNK
    ext = np.zeros((NSLOT, D), np.float32)
    ext[0:HALO] = x[b, start - HALO:start] if j > 0 else x[b, 0:HALO]
    ext[HALO:HALO + CHUNK] = x[b, start:start + CHUNK]
    ext[640] = x[b, 0]
    ext[641] = x[b, T - 1]
    return ext


def _make_mask(c):
    b, j = divmod(c, 4)
    start = j * CHUNK
    m = np.full((NQB, 128, WIN + 1), NEG, np.float32)
    il = np.arange(128)[:, None]
    jl = np.arange(WIN)[None, :]
    for qb in range(NQB):
        q_abs = start + qb * 128 + il
        slot = qb * 128 + jl
        band = (jl >= il) & (jl <= il + 128)
        valid = (j > 0) | (slot >= HALO)
        blk = m[qb, :, :WIN]
        blk[band & valid] = 0.0
        tok0_in_band = (q_abs[:, 0] <= HALO) & (j == 0)
        m[qb, :, WIN] = np.where(tok0_in_band, NEG, 0.0)
    return m


def _tileP(a, p=128):
    """[N*p, ...] -> [p, N, ...] partition-tiled layout."""
    n = a.shape[0] // p
    return np.ascontiguousarray(
        a.reshape(n, p, *a.shape[1:]).transpose(1, 0, *range(2, a.ndim + 1)))


def _vec_t(v):
    return np.ascontiguousarray(np.asarray(v, np.float32).reshape(-1, 128).T)

# ------------------------------------------------------------ bass program --


def _build_nc():
    nc = bass.Bass()

    inp = {}
    for name, shape, dt in [
        ("xT", [128, 8, NSLOT], BF16),
        ("wq", [128, 8, D], FP8), ("wk", [128, 8, D], FP8),
        ("wv", [128, 8, D], FP8), ("wo", [128, 8, D], FP8),
        ("w1", [128, 20, 8, 128], BF16), ("w2", [128, 20, D], BF16),
        ("w18", [128, 8, 1536], FP8), ("w28", [128, 12, D], FP8),
        ("msk", [128, NQB, WIN + 1], BF16),
        ("pblob", [128, 84], F32),
        ("fixsel", [128, 1], mybir.dt.uint8),
    ]:
        inp[name] = nc.dram_tensor(name, shape, dt, kind="ExternalInput")
    out_d = nc.dram_tensor("outT", [128, 8, CHUNK], F32, kind="ExternalOutput")
    pin = nc.dram_tensor("pin", [H, 2, HD + 1], F32)
    pout = nc.dram_tensor("pout", [H, 2, HD + 1], F32, addr_space="Shared")

    with tile.TileContext(nc) as tc:
        _emit(nc, tc, inp, out_d, pin, pout)
    _split_multiwaits(nc)
    return nc


def _emit(nc, tc, inp, out_d, pin, pout):
    from contextlib import ExitStack
    ctx = ExitStack()
    with ctx:
        pers = ctx.enter_context(tc.tile_pool(name="pers", bufs=1))
        small = ctx.enter_context(tc.tile_pool(name="small", bufs=3))
        bigY = ctx.enter_context(tc.tile_pool(name="bigY", bufs=1))  # yT

        # ---- persistent constants / params
        idf = pers.tile([128, 128], F32, tag="idf")
        make_identity(nc, idf)
        idb = pers.tile([128, 128], BF16, tag="idb")
        make_identity(nc, idb)
        onesD = pers.tile([128, 1], BF16, tag="onesD")   # 1/D for means
        nc.vector.memset(onesD, 1.0 / D)
        onesDf = pers.tile([128, 1], F32, tag="onesDf")
        nc.vector.memset(onesDf, 1.0 / D)
        ones8 = pers.tile([128, 1], FP8, tag="ones8")
        nc.vector.memset(ones8, 1.0)
        ones16 = pers.tile([1, 128], F32, tag="ones16")  # LN1 bcast (x16 fp8)
        nc.vector.memset(ones16, SH)
        ones1f = pers.tile([1, 128], F32, tag="ones1f")  # LN2 bcast
        nc.vector.memset(ones1f, 1.0)
        epst = pers.tile([1, 1], F32, tag="epst")
        nc.vector.memset(epst, EPS)
        neg3 = pers.tile([128, 1], F32, tag="neg3")
        nc.vector.memset(neg3, -3.0)

        yT = bigY.tile([128, 8, CHUNK], F32, tag="yT")

        # ================= LN in transposed layout =========================
        def layernorm_T(src, xbt, width, nchunks, g, b, ones_bc, out, pools,
                        apply_width=None):
            apply_width = apply_width or width
            ps_row, ps_bc = pools
            cw = width // nchunks
            mus = []
            for cch in range(nchunks):
                mus.append((ps_row.tile([1, cw], F32, tag="row", name="mu"),
                            ps_row.tile([1, cw], F32, tag="row", name="msq")))
            for kt in range(8):
                xsq = small.tile([128, width], BF16, tag="ln_xsq")
                if xbt is None:
                    xb = src[:, kt, 0:width]
                    # bf16 source: square on DVE (2x mode), Act stays free
                    nc.vector.tensor_mul(out=xsq, in0=xb, in1=xb)
                else:
                    xb = xbt[:, kt, 0:width]
                    # cast copy split DVE/Pool; feeds only the apply stage
                    if kt % 2 == 0:
                        nc.vector.tensor_copy(out=xb, in_=src[:, kt, :])
                    else:
                        nc.gpsimd.tensor_copy(out=xb, in_=src[:, kt, :])
                    nc.scalar.square(out=xsq, in_=src[:, kt, :])
                for cch in range(nchunks):
                    sl = slice(cch * cw, (cch + 1) * cw)
                    nc.tensor.matmul(mus[cch][0], onesD, xb[:, sl],
                                     start=kt == 0, stop=kt == 7)
                    nc.tensor.matmul(mus[cch][1], onesD, xsq[:, sl],
                                     start=kt == 0, stop=kt == 7)
            bcs = []
            for cch in range(nchunks):
                mu_ps, msq_ps = mus[cch]
                musb = small.tile([1, cw], F32, tag="ln_mu")
                nc.scalar.copy(out=musb, in_=mu_ps)
                tmp = small.tile([1, cw], F32, tag="ln_tmp")
                nc.vector.tensor_mul(out=tmp, in0=musb, in1=musb)
                nc.vector.tensor_sub(out=tmp, in0=msq_ps, in1=tmp)
                nc.scalar.activation(out=tmp, in_=tmp, func=AF.Sqrt,
                                     bias=epst, scale=1.0)
                nc.vector.reciprocal(out=tmp, in_=tmp)       # rstd
                nc.vector.tensor_mul(out=musb, in0=musb, in1=tmp)
                nc.scalar.mul(out=musb, in_=musb, mul=-1.0)  # -mu*rstd
                rb_ps = ps_bc.tile([128, cw], F32, tag="bc", name="rb")
                nc.tensor.matmul(rb_ps, ones_bc, tmp, start=True, stop=True)
                nb_ps = ps_bc.tile([128, cw], F32, tag="bc", name="nb")
                nc.tensor.matmul(nb_ps, ones_bc, musb, start=True, stop=True)
                rb_sb = small.tile([128, cw], BF16, tag="ln_rb")
                nc.scalar.copy(out=rb_sb, in_=rb_ps)
                nb_sb = small.tile([128, cw], BF16, tag="ln_nb")
                nc.scalar.copy(out=nb_sb, in_=nb_ps)
                bcs.append((rb_sb, nb_sb))
            for kt in range(8):
                for cch in range(nchunks):
                    lo, hi = cch * cw, min((cch + 1) * cw, apply_width)
                    if hi <= lo:
                        continue
                    w = hi - lo
                    sl = slice(lo, hi)
                    rb_sb, nb_sb = bcs[cch]
                    src_kt = (src[:, kt, :] if xbt is None
                              else xbt[:, kt, 0:width])
                    t1 = small.tile([128, cw], BF16, tag="ln_t1")
                    t1 = t1[:, 0:w]
                    nc.vector.tensor_mul(out=t1, in0=src_kt[:, sl],
                                         in1=rb_sb[:, 0:w])
                    nc.vector.tensor_add(out=t1, in0=t1, in1=nb_sb[:, 0:w])
                    # g,b apply + dtype cast on Act
                    nc.scalar.activation(out=out[:, kt, sl], in_=t1,
                                         func=AF.Identity,
                                         bias=b[:, kt:kt + 1],
                                         scale=g[:, kt:kt + 1])

        with tc.tile_pool(name="bigG", bufs=1) as bigG, \
             tc.tile_pool(name="poolW", bufs=1) as poolW, \
             tc.tile_pool(name="poolB", bufs=1) as poolB:
            # xT first in the DMA queue (LN1 is the startup critical path)
            xT = bigG.tile([128, 8, NSLOT], BF16, tag="xT")
            for kt in range(8):
                nc.sync.dma_start(out=xT[:, kt, :], in_=inp["xT"][:, kt, :])

            pblob = pers.tile([128, 84], F32, tag="pblob")
            nc.sync.dma_start(out=pblob, in_=inp["pblob"][:])
            # packed params: [g1 b1 g2 b2 bo bo2](8 each) b1h(32) fA fB
            g1T = pblob[:, 0:8]
            b1T = pblob[:, 8:16]      # pre-scaled x16 on host
            g2T = pblob[:, 16:24]
            b2T = pblob[:, 24:32]
            boT = pblob[:, 32:40]
            bo2T = pblob[:, 40:48]
            b1h = pblob[:, 48:80]
            fA = pblob[0:16, 80:81]
            fB = pblob[0:16, 81:82]
            fixsel = pers.tile([128, 1], mybir.dt.uint8, tag="fixsel")
            nc.sync.dma_start(out=fixsel, in_=inp["fixsel"][:])
            msk = pers.tile([128, NQB, WIN + 1], BF16, tag="msk")
            nc.sync.dma_start(out=msk, in_=inp["msk"][:])

            # fp8 projection weights (whole tensors, one DMA each)
            wq8 = poolW.tile([128, 8, D], FP8, tag="wq8")
            nc.sync.dma_start(out=wq8, in_=inp["wq"][:])
            wk8 = poolW.tile([128, 8, D], FP8, tag="wk8")
            nc.sync.dma_start(out=wk8, in_=inp["wk"][:])
            wv8 = poolW.tile([128, 8, D], FP8, tag="wv8")
            nc.sync.dma_start(out=wv8, in_=inp["wv"][:])
            wo8 = bigG.tile([128, 8, D], FP8, tag="wo8")
            nc.sync.dma_start(out=wo8, in_=inp["wo"][:])

            hT8 = poolB.tile([128, 8, NSLOT], FP8, tag="hT8")
            QT = bigG.tile([128, 8, CHUNK], BF16, tag="QT")
            q47T = bigG.tile([128, 8], BF16, tag="q47T")
            KT = bigG.tile([128, 8, NKV], BF16, tag="KT")
            V8 = bigG.tile([128, 5, D], FP8, tag="V8")    # x16 scale
            v0r = bigG.tile([1, D], BF16, tag="v0r")      # x16 scale
            # zero-padded per-(sub,pr) copies of v0 so the rank-1 token-0
            # update can always target PSUM partitions 0..127 (walrus
            # rejects K=1 matmuls with dst partition base 64)
            v0z = bigG.tile([1, 2, 8, 128], BF16, tag="v0z")
            OT8 = bigG.tile([128, 8, CHUNK], FP8, tag="OT8")  # x16 scale

            _mark(nc, "B:ln1")
            # ================ Phase B: LN1 -> hT8 (x16, fp8) ===============
            with tc.tile_pool(name="ps_row1", bufs=4, space="PSUM") as psr, \
                 tc.tile_pool(name="ps_bc1", bufs=4, space="PSUM") as psb:
                layernorm_T(xT, None, NSLOT, 2, g1T, b1T, ones16, hT8,
                            (psr, psb), apply_width=642)

            _mark(nc, "C:qkv")
            # ========= Phase C: QKV fp8 DoubleRow (+ q2047, v0 row) ========
            DQ = 1.0 / (SW * SH)
            with tc.tile_pool(name="ps_big", bufs=6, space="PSUM") as ps_big, \
                 tc.tile_pool(name="ps_tiny", bufs=2, space="PSUM") as ps_tiny:
                for m in range(8):
                    msl = slice(m * 128, (m + 1) * 128)
                    q_ps = ps_big.tile([128, CHUNK], F32, tag="big")
                    q47_ps = ps_tiny.tile([128, 1], F32, tag="tiny")
                    for j in range(4):
                        pr = slice(2 * j, 2 * j + 2)
                        nc.tensor.matmul(q_ps, wq8[:, pr, msl],
                                         hT8[:, pr, HALO:HALO + CHUNK],
                                         start=j == 0, stop=j == 3,
                                         perf_mode=DR)
                        nc.tensor.matmul(q47_ps, wq8[:, pr, msl],
                                         hT8[:, pr, 641:642],
                                         start=j == 0, stop=j == 3,
                                         perf_mode=DR)
                    nc.scalar.mul(out=QT[:, m, :], in_=q_ps,
                                  mul=DQ / np.sqrt(HD))
                    nc.scalar.mul(out=q47T[:, m:m + 1], in_=q47_ps,
                                  mul=DQ / np.sqrt(HD))
                    k_ps = ps_big.tile([128, 512], F32, tag="big")
                    k_ps2 = ps_big.tile([128, NKV - 512], F32, tag="big")
                    for j in range(4):
                        pr = slice(2 * j, 2 * j + 2)
                        nc.tensor.matmul(k_ps, wk8[:, pr, msl],
                                         hT8[:, pr, 0:512],
                                         start=j == 0, stop=j == 3,
                                         perf_mode=DR)
                        nc.tensor.matmul(k_ps2, wk8[:, pr, msl],
                                         hT8[:, pr, 512:NKV],
                                         start=j == 0, stop=j == 3,
                                         perf_mode=DR)
                    nc.vector.tensor_scalar_mul(out=KT[:, m, 0:512],
                                                in0=k_ps, scalar1=DQ)
                    nc.vector.tensor_scalar_mul(out=KT[:, m, 512:NKV],
                                                in0=k_ps2, scalar1=DQ)
                for tt in range(5):
                    for cch in range(2):
                        csl = slice(cch * 512, (cch + 1) * 512)
                        v_ps = ps_big.tile([128, 512], F32, tag="big")
                        for j in range(4):
                            pr = slice(2 * j, 2 * j + 2)
                            nc.tensor.matmul(
                                v_ps, hT8[:, pr, tt * 128:(tt + 1) * 128],
                                wv8[:, pr, csl],
                                start=j == 0, stop=j == 3, perf_mode=DR)
                        # V8 holds 16*v (fp8)
                        nc.scalar.mul(out=V8[:, tt, csl], in_=v_ps,
                                      mul=DQ * SH)
                for cch in range(2):
                    csl = slice(cch * 512, (cch + 1) * 512)
                    v0_ps = ps_tiny.tile([1, 512], F32, tag="tiny")
                    for j in range(4):
                        pr = slice(2 * j, 2 * j + 2)
                        nc.tensor.matmul(v0_ps, hT8[:, pr, 640:641],
                                         wv8[:, pr, csl],
                                         start=j == 0, stop=j == 3,
                                         perf_mode=DR)
                    nc.scalar.mul(out=v0r[:, csl], in_=v0_ps, mul=DQ * SH)
                nc.vector.memset(v0z, 0.0)
                v0v = v0r.rearrange("p (h c) -> p h c", c=128)
                nc.vector.tensor_copy(out=v0z[:, 0, :, 0:64],
                                      in_=v0v[:, :, 0:64])
                nc.vector.tensor_copy(out=v0z[:, 1, :, 64:128],
                                      in_=v0v[:, :, 64:128])

                _mark(nc, "D:partials")
                # ========= Phase D: global-row partials + AllReduce ========
                s47_ps = ps_tiny.tile([128, H * 4], F32, tag="tiny")
                for h in range(H):
                    p0 = 64 * (h % 2)
                    for i in range(4):
                        nc.tensor.matmul(
                            s47_ps[:, 4 * h + i:4 * h + i + 1],
                            KT[p0:p0 + 64, h // 2,
                               HALO + 128 * i:HALO + 128 * (i + 1)],
                            q47T[p0:p0 + 64, h // 2:h // 2 + 1],
                            start=True, stop=True)
                p47 = small.tile([128, H * 4], FP8, tag="p_p47")
                nc.scalar.activation(out=p47, in_=s47_ps, func=AF.Exp)
                ssum_ps = ps_tiny.tile([1, H * 4], F32, tag="tiny")
                nc.tensor.matmul(ssum_ps, ones8, p47, start=True, stop=True)
                s_c = small.tile([1, H], F32, tag="p_sc")
                nc.vector.reduce_sum(
                    out=s_c, in_=ssum_ps.rearrange("p (h i) -> p h i", i=4),
                    axis=AX.X)
                oall = small.tile([65, H], F32, tag="p_oall")
                o47_ps = ps_tiny.tile([64, H], F32, tag="tiny")
                for h in range(H):
                    for i in range(4):
                        # V8 is 16*v: o partial comes out 16x, matching the
                        # x16 fp8 output scale of the patch column.
                        nc.tensor.matmul(o47_ps[:, h:h + 1],
                                         V8[:, 1 + i, 64 * h:64 * h + 64],
                                         p47[:, 4 * h + i:4 * h + i + 1],
                                         start=i == 0, stop=i == 3)
                nc.scalar.copy(out=oall[0:64, :], in_=o47_ps)
                nc.sync.dma_start(out=oall[64:65, :], in_=s_c)
                part_ps = ps_tiny.tile([H, 65], F32, tag="tiny")
                nc.tensor.transpose(part_ps, oall, idf[0:65, 0:65])
                part_sb = small.tile([H, 65], F32, tag="p_part")
                nc.scalar.copy(out=part_sb, in_=part_ps)
                pa = small.tile([H, 2, 65], F32, tag="p_pa")
                nc.vector.tensor_scalar_mul(out=pa[:, 0, :], in0=part_sb,
                                            scalar1=fA)
                nc.vector.tensor_scalar_mul(out=pa[:, 1, :], in0=part_sb,
                                            scalar1=fB)
                nc.sync.dma_start(out=pin[:], in_=pa)
                if not SKIP_CC[0]:
                    nc.gpsimd.collective_compute(
                        "AllReduce", ALU.add,
                        replica_groups=[[0, 1, 2, 3, 4, 5, 6, 7]],
                        ins=[pin[:]], outs=[pout[:]])
                gath = small.tile([H, 2, 65], F32, tag="p_gath")
                nc.sync.dma_start(out=gath,
                                  in_=(pin if SKIP_CC[0] else pout)[:])
                vA = small.tile([H, 65], F32, tag="p_vA")
                nc.vector.tensor_scalar_mul(out=vA, in0=gath[:, 0, :],
                                            scalar1=fA)
                vB = small.tile([H, 65], F32, tag="p_vB")
                nc.vector.tensor_scalar_mul(out=vB, in0=gath[:, 1, :],
                                            scalar1=fB)
                val = small.tile([H, 65], F32, tag="p_val")
                nc.vector.tensor_add(out=val, in0=vA, in1=vB)
                recS = small.tile([H, 1], F32, tag="p_recS")
                nc.vector.reciprocal(out=recS, in_=val[:, 64:65])
                a47 = small.tile([H, HD], F32, tag="p_a47")
                # o partial is 16x -> a47 lands at the x16 fp8 scale directly
                nc.vector.tensor_scalar_mul(out=a47, in0=val[:, 0:64],
                                            scalar1=recS)
                a47t_ps = ps_tiny.tile([HD, H], F32, tag="tiny")
                nc.tensor.transpose(a47t_ps, a47, idf[0:H, 0:H])
                a47T = small.tile([HD, H], FP8, tag="p_a47T")
                nc.scalar.copy(out=a47T, in_=a47t_ps)
                fix_sb = small.tile([128, 8], FP8, tag="p_fix")
                a47v = a47T.rearrange("p (t two) -> p t two", two=2)
                nc.sync.dma_start(out=fix_sb[0:64, :], in_=a47v[:, :, 0])
                nc.sync.dma_start(out=fix_sb[64:128, :], in_=a47v[:, :, 1])
                return fix_sb

            _mark(nc, "E:attn")
            # ================ Phase E: windowed attention ==================
            with tc.tile_pool(name="ps_s", bufs=2, space="PSUM") as ps_s, \
                 tc.tile_pool(name="ps_pt", bufs=3, space="PSUM") as ps_pt, \
                 tc.tile_pool(name="ps_p0", bufs=1, space="PSUM") as ps_p0, \
                 tc.tile_pool(name="ps_o", bufs=2, space="PSUM") as ps_o:
                for pr in range(8):
                    for qb in range(NQB):
                        o_ps = ps_o.tile([128, 128], F32, tag="o")
                        ptbs, pt0s = [], []
                        for sub in range(2):
                            p0 = 64 * sub
                            qs = QT[p0:p0 + 64, pr, qb * 128:(qb + 1) * 128]
                            s_ps = ps_s.tile([128, WIN + 1], F32, tag="s")
                            nc.tensor.matmul(
                                s_ps[:, 0:WIN], qs,
                                KT[p0:p0 + 64, pr, qb * 128:qb * 128 + WIN],
                                start=True, stop=False)
                            nc.tensor.matmul(s_ps[:, WIN:WIN + 1], qs,
                                             KT[p0:p0 + 64, pr, 640:641],
                                             start=False, stop=False)
                            nc.tensor.matmul(s_ps, idb, msk[:, qb, :],
                                             start=False, stop=True)
                            p = small.tile([128, WIN + 1], BF16, tag="a_p")
                            rsum = small.tile([128, 1], F32, tag="a_rsum")
                            if sub == 0:
                                nc.scalar.activation(out=p, in_=s_ps,
                                                     func=AF.Exp,
                                                     bias=neg3, scale=1.0,
                                                     accum_out=rsum)
                            else:
                                nc.scalar.activation(out=p, in_=s_ps,
                                                     func=AF.Exp,
                                                     bias=neg3, scale=1.0)
                                nc.vector.reduce_sum(out=rsum, in_=p,
                                                     axis=AX.X)
                            recip = small.tile([128, 1], F32, tag="a_recip")
                            nc.vector.reciprocal(out=recip, in_=rsum)
                            p2 = small.tile([128, WIN + 1], BF16, tag="a_p2")
                            nc.vector.tensor_scalar(out=p2, in0=p,
                                                    scalar1=recip, scalar2=SH,
                                                    op0=ALU.mult, op1=ALU.mult)
                            pt_ps = ps_pt.tile([128, WIN], BF16, tag="pt")
                            nc.tensor.transpose(pt_ps[:, 0:128], p2[:, 0:128],
                                                idb)
                            nc.tensor.transpose(pt_ps[:, 128:256],
                                                p2[:, 128:256], idb)
                            ptb = small.tile([128, WIN], BF16,
                                             tag="a_ptb8" if sub == 0
                                             else "a_ptbb")
                            if sub == 0:
                                nc.scalar.copy(out=ptb, in_=pt_ps)
                            else:
                                nc.vector.tensor_copy(out=ptb, in_=pt_ps)
                            pt0_ps = ps_p0.tile([1, 128], BF16, tag="pt0")
                            nc.tensor.transpose(pt0_ps, p2[:, WIN:WIN + 1],
                                                idb)
                            pt0b = small.tile([1, 128], BF16, tag="a_pt0b")
                            nc.vector.tensor_copy(out=pt0b, in_=pt0_ps)
                            ptbs.append(ptb)
                            pt0s.append(pt0b)
                        # (16p)@(16v) via DoubleRow (one instruction per
                        # sub-head, own region+group). Token-0 rank-1 updates
                        # go to a separate PSUM group (walrus rejects mixing
                        # DoubleRow and plain matmuls in one group) and are
                        # merged during evacuation.
                        # all-plain PV (fp8 stationary x bf16 moving); the
                        # token-0 rank-1 updates join the same accumulation,
                        # so o_ps = 256*o in one group
                        for sub in range(2):
                            h2s = 2 * pr + sub
                            p0 = 64 * sub
                            dv = slice(64 * h2s, 64 * h2s + 64)
                            nc.tensor.matmul(o_ps[p0:p0 + 64, :],
                                             V8[:, qb, dv],
                                             ptbs[sub][:, 0:128],
                                             start=True, stop=False,
                                             skip_group_check=True)
                            nc.tensor.matmul(o_ps[p0:p0 + 64, :],
                                             V8[:, qb + 1, dv],
                                             ptbs[sub][:, 128:256],
                                             start=False, stop=False,
                                             skip_group_check=True)
                        for sub in range(2):
                            nc.tensor.matmul(o_ps, v0z[:, sub, pr, :],
                                             pt0s[sub],
                                             start=False, stop=sub == 1,
                                             skip_group_check=True)
                        nc.vector.tensor_scalar_mul(
                            out=OT8[:, pr, qb * 128:(qb + 1) * 128],
                            in0=o_ps, scalar1=1.0 / SH)

            _mark(nc, "F:patch")
            # ================ Phase F: patch global row ====================
            for t in range(8):
                nc.vector.copy_predicated(out=OT8[:, t, CHUNK - 1:CHUNK],
                                          mask=fixsel,
                                          data=fix_sb[:, t:t + 1])

            # residual + bo precomputed off the critical path (runs under
            # the attention phase wall)
            xTb = bigG.tile([128, 8, CHUNK], F32, tag="xTb")
            for m in range(8):
                nc.gpsimd.tensor_scalar_add(out=xTb[:, m, :],
                                            in0=xT[:, m, HALO:HALO + CHUNK],
                                            scalar1=boT[:, m:m + 1])

            _mark(nc, "G:wo")
            # =========== Phase G: out-proj fp8 DR + residual ===============
            with tc.tile_pool(name="ps_g", bufs=4, space="PSUM") as ps_g:
                for m in range(8):
                    msl = slice(m * 128, (m + 1) * 128)
                    pr_ps = ps_g.tile([128, CHUNK], F32, tag="g")
                    for j in range(4):
                        prj = slice(2 * j, 2 * j + 2)
                        nc.tensor.matmul(pr_ps, wo8[:, prj, msl],
                                         OT8[:, prj, :],
                                         start=j == 0, stop=j == 3,
                                         perf_mode=DR)
                    nc.vector.scalar_tensor_tensor(
                        out=yT[:, m, :], in0=pr_ps, scalar=DQ,
                        in1=xTb[:, m, :], op0=ALU.mult, op1=ALU.add)

        # bigG/poolW/poolB closed: attention-side SBUF freed for the FFN
        with tc.tile_pool(name="poolF", bufs=1) as poolF, \
             tc.tile_pool(name="w1p", bufs=2) as w1p:
            # W2 resident before Phase J (DMA runs under LN2 + FFN1)
            # first two FFN1 weight batches go ahead of the W2 prefetch in
            # the DMA queue (W2 is not needed until Phase J)
            w1ts = []
            for mb in range(2):
                w1t = w1p.tile([128, 4, 8, 128], BF16, tag="w1t")
                nc.sync.dma_start(out=w1t,
                                  in_=inp["w1"][:, 4 * mb:4 * mb + 4])
                w1ts.append(w1t)
            w2sb = poolF.tile([128, 20, D], BF16, tag="w2sb")
            for i in range(3):
                ksl = slice(7 * i, min(7 * i + 7, 20))
                nc.sync.dma_start(out=w2sb[:, ksl, :], in_=inp["w2"][:, ksl, :])
            w28sb = poolF.tile([128, 12, D], FP8, tag="w28sb")
            nc.sync.dma_start(out=w28sb, in_=inp["w28"][:])
            w18sb = poolF.tile([128, 8, 1536], FP8, tag="w18sb")
            nc.sync.dma_start(out=w18sb, in_=inp["w18"][:])
            h2T = poolF.tile([128, 8, CHUNK], BF16, tag="h2T")
            h2T8 = poolF.tile([128, 8, CHUNK], FP8, tag="h2T8")
            htsb = poolF.tile([128, 20, CHUNK], BF16, tag="htsb")
            htsb8 = poolF.tile([128, 12, CHUNK], FP8, tag="htsb8")
            xbt2 = poolF.tile([128, 8, CHUNK], BF16, tag="ln_xb2")

            _mark(nc, "H:ln2")
            # ================= Phase H: LN2 ================================
            with tc.tile_pool(name="ps_row2", bufs=2, space="PSUM") as psr2, \
                 tc.tile_pool(name="ps_bc2", bufs=2, space="PSUM") as psb2:
                layernorm_T(yT, xbt2, CHUNK, 1, g2T, b2T, ones1f, h2T,
                            (psr2, psb2))

            # fp8 twin of h2 (x16) for the fp8 hidden blocks
            for kt in range(8):
                nc.vector.tensor_scalar_mul(out=h2T8[:, kt, :],
                                            in0=h2T[:, kt, :], scalar1=SH)

            _mark(nc, "I:ffn1")
            # ========= Phase I: FFN1 + gelu (hidden stays in SBUF) =========
            with tc.tile_pool(name="ps_f1", bufs=4, space="PSUM") as ps_f1, \
                 tc.tile_pool(name="ps_f18", bufs=2, space="PSUM") as ps_f18:
                for mb in range(5):
                    if mb < 2:
                        w1t = w1ts[mb]
                    else:
                        w1t = w1p.tile([128, 4, 8, 128], BF16, tag="w1t")
                        nc.sync.dma_start(out=w1t,
                                          in_=inp["w1"][:, 4 * mb:4 * mb + 4])
                    for mi in range(4):
                        m = 4 * mb + mi
                        h_ps = ps_f1.tile([128, CHUNK], F32, tag="f1")
                        for kt in range(8):
                            nc.tensor.matmul(h_ps, w1t[:, mi, kt, :],
                                             h2T[:, kt, :],
                                             start=kt == 0, stop=kt == 7)
                        nc.scalar.activation(out=htsb[:, m, :], in_=h_ps,
                                             func=AF.Gelu,
                                             bias=b1h[:, m:m + 1], scale=1.0)
                # last 8 hidden blocks in fp8 DoubleRow (x16 h2, x2048 W1);
                # gelu output stored fp8 at real scale
                for mi in range(12):
                    h_ps = ps_f18.tile([128, CHUNK], F32, tag="f18")
                    for j in range(4):
                        prj = slice(2 * j, 2 * j + 2)
                        nc.tensor.matmul(h_ps, w18sb[:, prj,
                                                     mi * 128:(mi + 1) * 128],
                                         h2T8[:, prj, :],
                                         start=j == 0, stop=j == 3,
                                         perf_mode=DR)
                    nc.scalar.activation(out=htsb8[:, mi, :], in_=h_ps,
                                         func=AF.Gelu,
                                         bias=b1h[:, 20 + mi:21 + mi],
                                         scale=DQ)

            _mark(nc, "J:ffn2")
            # ====== Phase J: FFN2 (m-outer, W2 resident) + residual ========
            with tc.tile_pool(name="ps_f2", bufs=2, space="PSUM") as ps_f2, \
                 tc.tile_pool(name="ps_f28", bufs=2, space="PSUM") as ps_f28:
                def f8_part(m, csl=slice(0, CHUNK)):
                    # fp8 contribution for output block m (hidden kt 24..31)
                    msl = slice(m * 128, (m + 1) * 128)
                    f28_ps = ps_f28.tile([128, CHUNK], F32, tag="f28")
                    for j in range(6):
                        prj = slice(2 * j, 2 * j + 2)
                        nc.tensor.matmul(f28_ps, w28sb[:, prj, msl],
                                         htsb8[:, prj, :],
                                         start=j == 0, stop=j == 5,
                                         perf_mode=DR)
                    e8 = small.tile([128, CHUNK], BF16, tag="e8")
                    nc.scalar.mul(out=e8, in_=f28_ps, mul=1.0 / SW)
                    return e8
                for m in range(7):
                    msl = slice(m * 128, (m + 1) * 128)
                    e8 = f8_part(m)
                    f2_ps = ps_f2.tile([128, CHUNK], F32, tag="f2")
                    for kt in range(20):
                        nc.tensor.matmul(f2_ps, w2sb[:, kt, msl],
                                         htsb[:, kt, :],
                                         start=kt == 0, stop=kt == 19)
                    om = small.tile([128, CHUNK], F32, tag="out_m")
                    nc.vector.scalar_tensor_tensor(
                        out=om, in0=f2_ps, scalar=bo2T[:, m:m + 1],
                        in1=e8, op0=ALU.add, op1=ALU.add)
                    nc.vector.tensor_add(out=om, in0=om, in1=yT[:, m, :])
                    nc.sync.dma_start(out=out_d[:, m, :], in_=om)
                # last m in four quarter-width groups so evacuation + DMA
                # drain under the remaining matmuls
                e8 = f8_part(7)
                for hf in range(4):
                    csl = slice(hf * 128, (hf + 1) * 128)
                    fh_ps = ps_f2.tile([128, 128], F32, tag="f2h")
                    for kt in range(20):
                        nc.tensor.matmul(fh_ps, w2sb[:, kt, 896:1024],
                                         htsb[:, kt, csl],
                                         start=kt == 0, stop=kt == 19)
                    om = small.tile([128, 128], F32, tag="out_mh")
                    nc.vector.scalar_tensor_tensor(
                        out=om, in0=fh_ps, scalar=bo2T[:, 7:8],
                        in1=e8[:, csl], op0=ALU.add, op1=ALU.add)
                    nc.vector.tensor_add(out=om, in0=om, in1=yT[:, 7, csl])
                    nc.sync.dma_start(out=out_d[:, 7, csl], in_=om)

# ------------------------------------------------------------------ driver --

_CACHE = {}


def _prep_core_inputs(inputs, c, shared_cache={}):
    bf = ml_dtypes.bfloat16
    f8 = ml_dtypes.float8_e4m3
    key = id(inputs.get("Wq"))
    shared = shared_cache.get(key)
    if shared is None:
        shared_cache.clear()

        def w8(w):
            return np.ascontiguousarray(
                (_tileP(np.asarray(w, np.float32)) * SW).astype(f8))
        pblob = np.zeros((128, 84), np.float32)
        pblob[:, 0:8] = _vec_t(inputs["ln1_g"])
        pblob[:, 8:16] = _vec_t(inputs["ln1_b"]) * SH
        pblob[:, 16:24] = _vec_t(inputs["ln2_g"])
        pblob[:, 24:32] = _vec_t(inputs["ln2_b"])
        pblob[:, 32:40] = _vec_t(inputs["bo"])
        pblob[:, 40:48] = _vec_t(inputs["b2"])
        pblob[:, 48:80] = np.asarray(inputs["b1"],
                                     np.float32).reshape(32, 128).T
        shared = {
            "wq": w8(inputs["Wq"]), "wk": w8(inputs["Wk"]),
            "wv": w8(inputs["Wv"]), "wo": w8(inputs["Wo"]),
            "w1": np.ascontiguousarray(
                np.asarray(inputs["W1"], np.float32)[:, :2560].astype(bf)
                .reshape(8, 128, 20, 128).transpose(1, 2, 0, 3)),
            "w18": w8(np.asarray(inputs["W1"], np.float32)[:, 2560:]),
            "w2": np.ascontiguousarray(
                np.asarray(inputs["W2"], np.float32)[:2560].astype(bf)
                .reshape(20, 128, D).transpose(1, 0, 2)),
            "w28": w8(np.asarray(inputs["W2"], np.float32)[2560:]),
            "pblob_base": pblob,
        }
        shared_cache[key] = shared
    x = np.asarray(inputs["x"], np.float32)
    xT = np.ascontiguousarray(
        _make_x_ext(x, c).T.reshape(8, 128, NSLOT)
        .transpose(1, 0, 2)).astype(bf)
    msk = np.ascontiguousarray(
        _make_mask(c).transpose(1, 0, 2)).astype(ml_dtypes.bfloat16)
    fs = np.full((128, 1), 1 if c % 4 == 3 else 0, np.uint8)
    pblob = shared["pblob_base"].copy()
    pblob[0:16, 80] = 1.0 if c < 4 else 0.0
    pblob[0:16, 81] = 0.0 if c < 4 else 1.0
    ret = {k: v for k, v in shared.items() if k != "pblob_base"}
    ret.update({"xT": xT, "msk": msk, "fixsel": fs, "pblob": pblob})
    return ret


def get_nc():
    if "nc" not in _CACHE:
        _CACHE["nc"] = _build_nc()
    return _CACHE["nc"]


def kernel(**inputs):
    nc = get_nc()
    in_maps = [_prep_core_inputs(inputs, c) for c in range(N_CORES)]
    res = run_bass_kernel_spmd(nc, in_maps, core_ids=list(range(N_CORES)),
                               trace=False)
    out = np.zeros((B, T, D), np.float32)
    for c in range(N_CORES):
        b, j = divmod(c, 4)
        oT = res.results[c]["outT"]          # [128, 8, 512]
        out[b, j * CHUNK:(j + 1) * CHUNK] = \
            oT.transpose(1, 0, 2).reshape(D, CHUNK).T
    return out
